# revision 13
# baseline (speedup 1.0000x reference)
"""Trainium2 Bass kernel for AdaptiveNeighbourSampling.

Row-parallel across 8 NeuronCores: each core owns 1024 rows of the
adjacency matrix, replicates the (normalized) feature matrix, computes its
sim block, edge-weighted probs and per-row top-16 (values + indices)
locally. No cross-core communication.

Per core, per 128-row tile:
  PE:   sim chunk = xnT_rows.T @ xnT          (fp32 matmuls, 512-wide)
  DVE:  w = sim * adj                          (tensor_tensor, PSUM src)
  ACT:  rowsum via Copy+accum (discard out); p = w * (1/rowsum)
  DVE:  hierarchical top-16: max8 per 512-seg -> L2 max8/match_replace/max8
        then max_index(vals, p) for global indices.
"""

import sys

if "/opt/trn_rl_repo" not in sys.path:
    sys.path.insert(0, "/opt/trn_rl_repo")

import numpy as np

import concourse.bass as bass
import concourse.tile as tile
from concourse import mybir
from concourse.bass_utils import run_bass_kernel_spmd
from concourse.masks import make_identity

N = 8192
D = 128
K = 16
NCORES = 8
R = N // NCORES          # rows per core
P = 128                  # partitions
T = R // P               # row tiles per core
CHUNK = 2048             # j-chunk for psum/mul
NCHUNK = N // CHUNK
SEG = 512                # L1 top-8 segment
NSEG = N // SEG
MMF = 512                # matmul moving free dim
F32 = mybir.dt.float32
U32 = mybir.dt.uint32
EPS = 1e-12
NEG = -3.0e38

AF = mybir.ActivationFunctionType


def split_waits(nc, max_waits=1):
    """Hoist surplus sync waits onto same-engine NoOps (this walrus build
    rejects instructions with more than one sync-wait command)."""
    total = 0
    for fn in nc.m.functions:
        for bb in fn.blocks:
            newlist = []
            for inst in bb.instructions:
                si = inst.sync_info
                if si is not None and len(si.on_wait) > max_waits:
                    waits = list(si.on_wait)
                    keep = waits[-max_waits:]
                    for wt in waits[:-max_waits]:
                        nop = mybir.InstNoOp(
                            name=f"I-ws-{nc.next_id()}", ins=[], outs=[]
                        )
                        nop.engine = inst.engine
                        nop.sync_info = mybir.SyncInfo(on_wait=[wt], on_update=[])
                        newlist.append(nop)
                        total += 1
                    inst.sync_info = mybir.SyncInfo(
                        on_wait=keep, on_update=list(si.on_update)
                    )
                newlist.append(inst)
            bb.instructions = newlist
    return total


def _normalize_transpose(nc, tc, sp, spp, src_ext, nrows, dstT, ident, tag):
    """Load [nrows,128] from DRAM, L2-normalize rows, write transposed
    [128, nrows] (features on partitions) into dstT."""
    nblk = nrows // P
    src_v = src_ext.rearrange("(b p) d -> p b d", p=P)
    for g in range(0, nblk, 8):
        xg = sp.tile([P, 8 * P], F32, name=f"xg_{tag}_{g}", tag="xg")
        nc.sync.dma_start(
            xg[:].rearrange("p (b d) -> p b d", d=P), src_v[:, g : g + 8, :]
        )
        n2 = sp.tile([P, 8], F32, name=f"n2_{tag}_{g}", tag="n2")
        for b in range(8):
            blk = xg[:, b * P : (b + 1) * P]
            nc.scalar.activation(
                n2[:, b : b + 1].broadcast_to([P, P]),
                blk,
                AF.Square,
                accum_out=n2[:, b : b + 1],
            )
        inv = sp.tile([P, 8], F32, name=f"inv_{tag}_{g}", tag="inv")
        nc.scalar.activation(inv[:], n2[:], AF.Sqrt)
        nc.vector.tensor_scalar_max(inv[:], inv[:], EPS)
        nc.vector.reciprocal(inv[:], inv[:])
        for b in range(8):
            xnb = sp.tile([P, P], F32, name=f"xnb_{tag}_{g}_{b}", tag="xnb")
            nc.vector.tensor_scalar_mul(
                xnb[:], xg[:, b * P : (b + 1) * P], inv[:, b : b + 1]
            )
            pt = spp.tile([P, P], F32, name=f"pt_{tag}_{g}_{b}", tag="pt")
            nc.tensor.transpose(pt[:], xnb[:], ident[:])
            nc.vector.tensor_copy(dstT[:, (g + b) * P : (g + b + 1) * P], pt[:])


def build():
    nc = bass.Bass()
    adj_ext = nc.declare_dram_parameter("adj", [R, N], F32, isOutput=False)
    xf_ext = nc.declare_dram_parameter("xf", [N, D], F32, isOutput=False)
    xr_ext = nc.declare_dram_parameter("xr", [R, D], F32, isOutput=False)
    vals_ext = nc.declare_dram_parameter("vals", [R, K], F32, isOutput=True)
    idx_ext = nc.declare_dram_parameter("idx", [R, K], U32, isOutput=True)

    with tile.TileContext(nc) as tc:
        with (
            tc.tile_pool(name="const", bufs=1) as constp,
            tc.tile_pool(name="io", bufs=2) as iop,
        ):
            ident = constp.tile([P, P], F32)
            make_identity(nc, ident[:])
            xfT = constp.tile([P, N], F32)
            xrT = constp.tile([P, R], F32)

            with (
                tc.tile_pool(name="setup", bufs=2) as sp,
                tc.tile_pool(name="setup_psum", bufs=4, space="PSUM") as spp,
            ):
                _normalize_transpose(nc, tc, sp, spp, xf_ext, N, xfT, ident, "f")
                _normalize_transpose(nc, tc, sp, spp, xr_ext, R, xrT, ident, "r")

            with (
                tc.tile_pool(name="adjp", bufs=6) as adjp,
                tc.tile_pool(name="simp", bufs=4) as simp,
                tc.tile_pool(name="wp", bufs=2) as wp,
                tc.tile_pool(name="smp", bufs=2) as smp,
                tc.tile_pool(name="psum", bufs=2, space="PSUM") as psp,
            ):
                pending = []

                def produce(t):
                    """Front half of tile t: DMA adj, matmuls, evac, mul,
                    rowsum partials. Returns state for the deferred half."""
                    w = wp.tile([P, N], F32, name=f"w_{t}", tag="w")
                    rs4 = smp.tile([P, NCHUNK], F32, name=f"rs4_{t}", tag="rs4")
                    lhsT = xrT[:, t * P : (t + 1) * P]
                    adj_cs = []
                    for c in range(NCHUNK):
                        ac = adjp.tile(
                            [P, CHUNK], F32, name=f"adj_{t}_{c}", tag="adj"
                        )
                        nc.sync.dma_start(
                            ac[:],
                            adj_ext[
                                t * P : (t + 1) * P,
                                c * CHUNK : (c + 1) * CHUNK,
                            ],
                        )
                        adj_cs.append(ac)
                    for c in range(NCHUNK):
                        ps = psp.tile([P, CHUNK], F32, name=f"sim_{t}_{c}", tag="sim")
                        for q in range(CHUNK // MMF):
                            nc.tensor.matmul(
                                ps[:, q * MMF : (q + 1) * MMF],
                                lhsT,
                                xfT[:, c * CHUNK + q * MMF : c * CHUNK + (q + 1) * MMF],
                                start=True,
                                stop=True,
                            )
                        # evacuate sim chunk PSUM->SBUF on ACT
                        sim_sb = simp.tile(
                            [P, CHUNK], F32, name=f"simsb_{t}_{c}", tag="simsb"
                        )
                        with tc.high_priority():
                            nc.scalar.activation(sim_sb[:], ps[:], AF.Copy)
                        wc = w[:, c * CHUNK : (c + 1) * CHUNK]
                        # weighted sim on GPSIMD (frees DVE for top-k work)
                        nc.gpsimd.tensor_mul(wc, sim_sb[:], adj_cs[c][:])
                        # per-chunk rowsum partial on ACT (overlaps next chunk's mul)
                        nc.scalar.activation(
                            rs4[:, c : c + 1].broadcast_to([P, CHUNK]),
                            wc,
                            AF.Copy,
                            accum_out=rs4[:, c : c + 1],
                        )
                    return (t, w, rs4)

                def finish(state):
                    """Back half of tile t: scale + top-16 + output. Emitted
                    after tile t+1's front half so ACT services t+1's PSUM
                    evacuations before the big scale pass (keeps PE fed)."""
                    t, w, rs4 = state
                    r = smp.tile([P, 1], F32, name=f"r_{t}", tag="r")
                    nc.vector.tensor_reduce(
                        r[:], rs4[:], axis=mybir.AxisListType.X, op=mybir.AluOpType.add
                    )
                    nc.vector.reciprocal(r[:], r[:])
                    # p = w * r, scaled in place (exact fl(w*r), ACT pass)
                    nc.scalar.activation(w[:], w[:], AF.Copy, scale=r[:])
                    # L1: top-8 per 512-segment
                    m8 = smp.tile([P, 8 * NSEG], F32, name=f"m8_{t}", tag="m8")
                    for s in range(NSEG):
                        nc.vector.max(
                            m8[:, s * 8 : (s + 1) * 8],
                            w[:, s * SEG : (s + 1) * SEG],
                        )
                    # L2: top-16 of the 128 candidates
                    v = smp.tile([P, K], F32, name=f"v_{t}", tag="v")
                    m8b = smp.tile([P, 8 * NSEG], F32, name=f"m8b_{t}", tag="m8b")
                    nc.vector.max(v[:, 0:8], m8[:])
                    nc.vector.match_replace(m8b[:], v[:, 0:8], m8[:], NEG)
                    nc.vector.max(v[:, 8:16], m8b[:])
                    ix = smp.tile([P, K], U32, name=f"ix_{t}", tag="ix")
                    nc.vector.max_index(ix[:, 0:8], v[:, 0:8], w[:])
                    nc.vector.max_index(ix[:, 8:16], v[:, 8:16], w[:])
                    nc.sync.dma_start(vals_ext[t * P : (t + 1) * P, :], v[:])
                    nc.sync.dma_start(idx_ext[t * P : (t + 1) * P, :], ix[:])

                for t in range(T):
                    st = produce(t)
                    if pending:
                        finish(pending.pop())
                    pending.append(st)
                finish(pending.pop())

    split_waits(nc)
    return nc


_NC_CACHE = None


def _get_nc():
    global _NC_CACHE
    if _NC_CACHE is None:
        _NC_CACHE = build()
    return _NC_CACHE


def kernel(adjacency_matrix, transaction_record, labels=None, k=None, **_unused):
    adj = np.ascontiguousarray(np.asarray(adjacency_matrix, dtype=np.float32))
    x = np.ascontiguousarray(np.asarray(transaction_record, dtype=np.float32))
    assert adj.shape == (N, N) and x.shape == (N, D)

    nc = _get_nc()
    in_maps = [
        {
            "adj": adj[i * R : (i + 1) * R],
            "xf": x,
            "xr": np.ascontiguousarray(x[i * R : (i + 1) * R]),
        }
        for i in range(NCORES)
    ]
    res = run_bass_kernel_spmd(nc, in_maps, core_ids=list(range(NCORES)))
    vals = np.concatenate([res.results[i]["vals"] for i in range(NCORES)], axis=0)
    idx = np.concatenate(
        [res.results[i]["idx"].astype(np.int32) for i in range(NCORES)], axis=0
    )
    return vals, idx


# revision 15
# speedup vs baseline: 1.0460x; 1.0460x over previous
"""Trainium2 Bass kernel for AdaptiveNeighbourSampling.

Row-parallel across 8 NeuronCores: each core owns 1024 rows of the
adjacency matrix, replicates the (normalized) feature matrix, computes its
sim block, edge-weighted probs and per-row top-16 (values + indices)
locally. No cross-core communication.

Per core, per 128-row tile:
  PE:   sim chunk = xnT_rows.T @ xnT          (fp32 matmuls, 512-wide)
  DVE:  w = sim * adj                          (tensor_tensor, PSUM src)
  ACT:  rowsum via Copy+accum (discard out); p = w * (1/rowsum)
  DVE:  hierarchical top-16: max8 per 512-seg -> L2 max8/match_replace/max8
        then max_index(vals, p) for global indices.
"""

import sys

if "/opt/trn_rl_repo" not in sys.path:
    sys.path.insert(0, "/opt/trn_rl_repo")

import numpy as np

import concourse.bass as bass
import concourse.tile as tile
from concourse import mybir
from concourse.bass_utils import run_bass_kernel_spmd
from concourse.masks import make_identity

N = 8192
D = 128
K = 16
NCORES = 8
R = N // NCORES          # rows per core
P = 128                  # partitions
T = R // P               # row tiles per core
CHUNK = 2048             # j-chunk for psum/mul
NCHUNK = N // CHUNK
SEG = 512                # L1 top-8 segment
NSEG = N // SEG
MMF = 512                # matmul moving free dim
F32 = mybir.dt.float32
U32 = mybir.dt.uint32
EPS = 1e-12
NEG = -3.0e38

AF = mybir.ActivationFunctionType


def split_waits(nc, max_waits=1):
    """Hoist surplus sync waits onto same-engine NoOps (this walrus build
    rejects instructions with more than one sync-wait command)."""
    total = 0
    for fn in nc.m.functions:
        for bb in fn.blocks:
            newlist = []
            for inst in bb.instructions:
                si = inst.sync_info
                if si is not None and len(si.on_wait) > max_waits:
                    waits = list(si.on_wait)
                    keep = waits[-max_waits:]
                    for wt in waits[:-max_waits]:
                        nop = mybir.InstNoOp(
                            name=f"I-ws-{nc.next_id()}", ins=[], outs=[]
                        )
                        nop.engine = inst.engine
                        nop.sync_info = mybir.SyncInfo(on_wait=[wt], on_update=[])
                        newlist.append(nop)
                        total += 1
                    inst.sync_info = mybir.SyncInfo(
                        on_wait=keep, on_update=list(si.on_update)
                    )
                newlist.append(inst)
            bb.instructions = newlist
    return total


def _normalize_transpose(nc, tc, sp, spp, src_ext, nrows, dstT, ident, tag):
    """Load [nrows,128] from DRAM, L2-normalize rows, write transposed
    [128, nrows] (features on partitions) into dstT."""
    nblk = nrows // P
    src_v = src_ext.rearrange("(b p) d -> p b d", p=P)
    for g in range(0, nblk, 8):
        xg = sp.tile([P, 8 * P], F32, name=f"xg_{tag}_{g}", tag="xg")
        nc.sync.dma_start(
            xg[:].rearrange("p (b d) -> p b d", d=P), src_v[:, g : g + 8, :]
        )
        n2 = sp.tile([P, 8], F32, name=f"n2_{tag}_{g}", tag="n2")
        for b in range(8):
            blk = xg[:, b * P : (b + 1) * P]
            nc.scalar.activation(
                n2[:, b : b + 1].broadcast_to([P, P]),
                blk,
                AF.Square,
                accum_out=n2[:, b : b + 1],
            )
        inv = sp.tile([P, 8], F32, name=f"inv_{tag}_{g}", tag="inv")
        nc.scalar.activation(inv[:], n2[:], AF.Sqrt)
        nc.vector.tensor_scalar_max(inv[:], inv[:], EPS)
        nc.vector.reciprocal(inv[:], inv[:])
        for b in range(8):
            xnb = sp.tile([P, P], F32, name=f"xnb_{tag}_{g}_{b}", tag="xnb")
            nc.vector.tensor_scalar_mul(
                xnb[:], xg[:, b * P : (b + 1) * P], inv[:, b : b + 1]
            )
            pt = spp.tile([P, P], F32, name=f"pt_{tag}_{g}_{b}", tag="pt")
            nc.tensor.transpose(pt[:], xnb[:], ident[:])
            nc.vector.tensor_copy(dstT[:, (g + b) * P : (g + b + 1) * P], pt[:])


def build():
    nc = bass.Bass()
    adj_ext = nc.declare_dram_parameter("adj", [R, N], F32, isOutput=False)
    xf_ext = nc.declare_dram_parameter("xf", [N, D], F32, isOutput=False)
    xr_ext = nc.declare_dram_parameter("xr", [R, D], F32, isOutput=False)
    vals_ext = nc.declare_dram_parameter("vals", [R, K], F32, isOutput=True)
    idx_ext = nc.declare_dram_parameter("idx", [R, K], U32, isOutput=True)

    with tile.TileContext(nc) as tc:
        with (
            tc.tile_pool(name="const", bufs=1) as constp,
            tc.tile_pool(name="io", bufs=2) as iop,
        ):
            ident = constp.tile([P, P], F32)
            make_identity(nc, ident[:])
            xfT = constp.tile([P, N], F32)
            xrT = constp.tile([P, R], F32)

            with (
                tc.tile_pool(name="setup", bufs=2) as sp,
                tc.tile_pool(name="setup_psum", bufs=4, space="PSUM") as spp,
            ):
                _normalize_transpose(nc, tc, sp, spp, xf_ext, N, xfT, ident, "f")
                _normalize_transpose(nc, tc, sp, spp, xr_ext, R, xrT, ident, "r")

            with (
                tc.tile_pool(name="adjp", bufs=6) as adjp,
                tc.tile_pool(name="simp", bufs=4) as simp,
                tc.tile_pool(name="wp", bufs=2) as wp,
                tc.tile_pool(name="smp", bufs=2) as smp,
                tc.tile_pool(name="psum", bufs=2, space="PSUM") as psp,
            ):
                pending = []

                def produce(t):
                    """Front half of tile t: DMA adj, matmuls, evac, mul.
                    Returns state for the deferred half."""
                    w = wp.tile([P, N], F32, name=f"w_{t}", tag="w")
                    lhsT = xrT[:, t * P : (t + 1) * P]
                    adj_cs = []
                    for c in range(NCHUNK):
                        ac = adjp.tile(
                            [P, CHUNK], F32, name=f"adj_{t}_{c}", tag="adj"
                        )
                        nc.sync.dma_start(
                            ac[:],
                            adj_ext[
                                t * P : (t + 1) * P,
                                c * CHUNK : (c + 1) * CHUNK,
                            ],
                        )
                        adj_cs.append(ac)
                    for c in range(NCHUNK):
                        ps = psp.tile([P, CHUNK], F32, name=f"sim_{t}_{c}", tag="sim")
                        for q in range(CHUNK // MMF):
                            nc.tensor.matmul(
                                ps[:, q * MMF : (q + 1) * MMF],
                                lhsT,
                                xfT[:, c * CHUNK + q * MMF : c * CHUNK + (q + 1) * MMF],
                                start=True,
                                stop=True,
                            )
                        # evacuate sim chunk PSUM->SBUF on ACT
                        sim_sb = simp.tile(
                            [P, CHUNK], F32, name=f"simsb_{t}_{c}", tag="simsb"
                        )
                        nc.scalar.activation(sim_sb[:], ps[:], AF.Copy)
                        wc = w[:, c * CHUNK : (c + 1) * CHUNK]
                        # weighted sim on GPSIMD (frees DVE for top-k work)
                        nc.gpsimd.tensor_mul(wc, sim_sb[:], adj_cs[c][:])
                    return (t, w)

                def finish(state):
                    """Back half of tile t: rowsum + scale + top-16 + output.
                    Emitted after tile t+1's front half, so every op here is
                    ready when ACT/DVE reach it (no head-of-line stalls in
                    front of t+1's PSUM evacuations)."""
                    t, w = state
                    rs = smp.tile([P, 1], F32, name=f"rs_{t}", tag="rs")
                    nc.scalar.activation(
                        rs[:].broadcast_to([P, N]), w[:], AF.Copy, accum_out=rs[:]
                    )
                    r = smp.tile([P, 1], F32, name=f"r_{t}", tag="r")
                    nc.vector.reciprocal(r[:], rs[:])
                    # p = w * r, scaled in place (exact fl(w*r), ACT pass)
                    nc.scalar.activation(w[:], w[:], AF.Copy, scale=r[:])
                    # L1: top-8 per 512-segment
                    m8 = smp.tile([P, 8 * NSEG], F32, name=f"m8_{t}", tag="m8")
                    for s in range(NSEG):
                        nc.vector.max(
                            m8[:, s * 8 : (s + 1) * 8],
                            w[:, s * SEG : (s + 1) * SEG],
                        )
                    # L2: top-16 of the 128 candidates
                    v = smp.tile([P, K], F32, name=f"v_{t}", tag="v")
                    m8b = smp.tile([P, 8 * NSEG], F32, name=f"m8b_{t}", tag="m8b")
                    nc.vector.max(v[:, 0:8], m8[:])
                    nc.vector.match_replace(m8b[:], v[:, 0:8], m8[:], NEG)
                    nc.vector.max(v[:, 8:16], m8b[:])
                    ix = smp.tile([P, K], U32, name=f"ix_{t}", tag="ix")
                    nc.vector.max_index(ix[:, 0:8], v[:, 0:8], w[:])
                    nc.vector.max_index(ix[:, 8:16], v[:, 8:16], w[:])
                    nc.sync.dma_start(vals_ext[t * P : (t + 1) * P, :], v[:])
                    nc.sync.dma_start(idx_ext[t * P : (t + 1) * P, :], ix[:])

                for t in range(T):
                    st = produce(t)
                    if pending:
                        finish(pending.pop())
                    pending.append(st)
                finish(pending.pop())

    split_waits(nc)
    return nc


_NC_CACHE = None


def _get_nc():
    global _NC_CACHE
    if _NC_CACHE is None:
        _NC_CACHE = build()
    return _NC_CACHE


def kernel(adjacency_matrix, transaction_record, labels=None, k=None, **_unused):
    adj = np.ascontiguousarray(np.asarray(adjacency_matrix, dtype=np.float32))
    x = np.ascontiguousarray(np.asarray(transaction_record, dtype=np.float32))
    assert adj.shape == (N, N) and x.shape == (N, D)

    nc = _get_nc()
    in_maps = [
        {
            "adj": adj[i * R : (i + 1) * R],
            "xf": x,
            "xr": np.ascontiguousarray(x[i * R : (i + 1) * R]),
        }
        for i in range(NCORES)
    ]
    res = run_bass_kernel_spmd(nc, in_maps, core_ids=list(range(NCORES)))
    vals = np.concatenate([res.results[i]["vals"] for i in range(NCORES)], axis=0)
    idx = np.concatenate(
        [res.results[i]["idx"].astype(np.int32) for i in range(NCORES)], axis=0
    )
    return vals, idx
